# revision 19
# baseline (speedup 1.0000x reference)
"""Bahdanau additive attention kernel for 8 Trainium2 NeuronCores.

Reference computation (per batch b):
  q = query @ W1                  [TQ, U]
  k = value @ W2                  [TV, U]
  scores[i,j] = sum_u scale[u] * tanh(q[i,u] + k[j,u])
  attn = softmax(scores, axis=-1)
  ctx = attn @ value

Sharding: the B*TQ = 2048 query rows split into 8 chunks of 256; core c
handles batch c//4, query rows (c%4)*256 .. +256.  Each core gets its
query chunk plus the full value/W1/W2/scale (data-parallel, no
collectives).

Per-core dataflow (U = 128 = partition count):
  - load query/value/W tiles (DMAs spread over three queues), convert to
    fp16 on DVE, PE-transpose the fp16 copies to get d-major layouts
    (fp16 keeps the PE off its 2-pass fp32 LOW_HIGH weight-load path;
    fp16 rounding of the projection inputs costs ~5e-4 relative)
  - kprojT [U, TV] and qprojT [U, QCH] via fp16 PE matmuls (fp32 psum)
  - per query i: ACT computes t = tanh(kprojT + qprojT[:, i]) in one
    [128, 1024] instruction (per-partition bias), output cast to fp16;
    8 PE matmuls with t as the stationary operand and scale [128,1]
    moving produce the score column for each 128-key tile
  - per 64-query half-tile: PE-transpose scoresT back (fp32, separate
    psum/sbuf tiles per half so the first half's softmax prep overlaps
    the tanh stream), batched softmax (DVE max -> ACT exp with
    accumulator -> DVE reciprocal -> DVE scale), fp16 attn copy for the
    context matmul against fp16 value tiles.  Both qtiles' tanh/score
    loops are emitted before any softmax so the ACT stream never stalls.
"""

from contextlib import ExitStack

import numpy as np

from concourse import bacc, bass, masks, mybir
import concourse.tile as tile
from concourse.bass_utils import run_bass_kernel_spmd

F32 = mybir.dt.float32
F16 = mybir.dt.float16
AF = mybir.ActivationFunctionType
ALU = mybir.AluOpType
AX = mybir.AxisListType

B, TQ, TV, D, U = 2, 1024, 1024, 512, 128
NCORES = 8
QCH = (B * TQ) // NCORES  # 256 query rows per core
NQT = QCH // 128          # 2 query tiles per core
NVT = TV // 128           # 8 value tiles
NDT = D // 128            # 4 d tiles


def build_program() -> bass.Bass:
    nc = bacc.Bacc(None, target_bir_lowering=False)
    # fp16 (and pre-transposed where needed) inputs, prepared host-side in
    # make_in_maps; the device consumes only these layouts.
    v16_in = nc.declare_dram_parameter("v16", [128, NVT, D], F16, isOutput=False)
    vT16_in = nc.declare_dram_parameter("vT16", [128, NDT, TV], F16, isOutput=False)
    qT16_in = nc.declare_dram_parameter("qT16", [128, NDT, QCH], F16, isOutput=False)
    w1_in = nc.declare_dram_parameter("w1_16", [128, NDT, U], F16, isOutput=False)
    w2_in = nc.declare_dram_parameter("w2_16", [128, NDT, U], F16, isOutput=False)
    s_in = nc.declare_dram_parameter("s16", [U, 1], F16, isOutput=False)
    attn_out = nc.declare_dram_parameter("attn", [QCH, TV], F32, isOutput=True)
    ctx_out = nc.declare_dram_parameter("ctx", [QCH, D], F32, isOutput=True)

    with tile.TileContext(nc) as tc, ExitStack() as ctx:
        const = ctx.enter_context(tc.tile_pool(name="const", bufs=1))
        sb = ctx.enter_context(tc.tile_pool(name="sb", bufs=1))
        tpool = ctx.enter_context(tc.tile_pool(name="tpool", bufs=4))
        qt_pool = ctx.enter_context(tc.tile_pool(name="qt_pool", bufs=2))
        half_pool = ctx.enter_context(tc.tile_pool(name="half_pool", bufs=4))
        small = ctx.enter_context(tc.tile_pool(name="small", bufs=8))
        # PSUM: scoresT half tiles are 1 bank each, 4 alive at once;
        # everything else shares one-bank slots.
        ps_sc = ctx.enter_context(tc.tile_pool(name="ps_sc", bufs=3, space="PSUM"))
        ps_tr = ctx.enter_context(tc.tile_pool(name="ps_tr", bufs=4, space="PSUM"))

        identity = const.tile([128, 128], F32)
        masks.make_identity(nc, identity[:])
        identity16 = const.tile([128, 128], F16)
        masks.make_identity(nc, identity16[:])

        s16_sb = const.tile([U, 1], F16)
        nc.scalar.dma_start(out=s16_sb[:], in_=s_in[:])

        # weight + transposed-operand loads gate the projections; the
        # non-transposed value copy (ctx matmul operand) is only needed at
        # the end of the stream, so it loads last on the sync queue.
        w2_16_sb = sb.tile([128, NDT, U], F16)
        nc.gpsimd.dma_start(out=w2_16_sb[:], in_=w2_in[:])
        valueT16_sb = sb.tile([128, NDT, TV], F16)
        nc.gpsimd.dma_start(out=valueT16_sb[:], in_=vT16_in[:])
        w1_16_sb = sb.tile([128, NDT, U], F16)
        nc.scalar.dma_start(out=w1_16_sb[:], in_=w1_in[:])
        queryT16_sb = sb.tile([128, NDT, QCH], F16)
        nc.scalar.dma_start(out=queryT16_sb[:], in_=qT16_in[:])
        value16_sb = sb.tile([128, NVT, D], F16)
        nc.sync.dma_start(out=value16_sb[:], in_=v16_in[:])

        # qprojT[u, i] = sum_d W1[d, u] * query[i, d]   (fp32 accumulate)
        qprojT_sb = sb.tile([U, QCH], F32)
        psq = ps_tr.tile([U, QCH], F32, tag="ps_tr")
        for d in range(NDT):
            nc.tensor.matmul(psq[:], w1_16_sb[:, d, :], queryT16_sb[:, d, :],
                             start=(d == 0), stop=(d == NDT - 1))
        nc.vector.tensor_copy(qprojT_sb[:], psq[:])

        # kprojT[u, j] = sum_d W2[d, u] * value[j, d]
        kprojT_sb = sb.tile([U, TV], F32)
        for h in range(2):
            psk = ps_tr.tile([U, 512], F32, tag="ps_tr")
            for d in range(NDT):
                nc.tensor.matmul(psk[:], w2_16_sb[:, d, :],
                                 valueT16_sb[:, d, h * 512:(h + 1) * 512],
                                 start=(d == 0), stop=(d == NDT - 1))
            nc.vector.tensor_copy(kprojT_sb[:, h * 512:(h + 1) * 512], psk[:])

        # ---- main stream, emitted per 64-query half-block with a 1-half
        # lag: PE executes its queue in order, so each half's softmax-prep
        # transposes are queued right after the NEXT half's score matmuls.
        # That way they run while ACT is still on the next half's tanh
        # stream, and each exp's dependency chain is already done when ACT
        # reaches it.  Only the last half's chain is exposed at the end.

        def emit_half_scores(qt, h):
            # sc_ps[m, vt, c] = scores[qt*128 + h*64 + c, vt*128 + m]
            sc_ps = ps_sc.tile([128, NVT, 64], F32, tag="sc_ps_half")
            for c in range(64):
                i = qt * 128 + h * 64 + c
                t_t = tpool.tile([U, TV], F16, tag="t_t")
                nc.scalar.activation(out=t_t[:], in_=kprojT_sb[:], func=AF.Tanh,
                                     bias=qprojT_sb[:, i:i + 1], scale=1.0)
                for vt in range(NVT):
                    nc.tensor.matmul(sc_ps[:, vt, c:c + 1],
                                     t_t[:, vt * 128:(vt + 1) * 128], s16_sb[:],
                                     start=True, stop=True)
            return sc_ps

        def emit_half_prep(qt, h, sc_ps):
            scoresT_sb = half_pool.tile([128, NVT, 64], F32, tag="scoresT")
            nc.vector.tensor_copy(scoresT_sb[:], sc_ps[:])
            scores_sb = half_pool.tile([64, TV], F32, tag="scores")
            for vt in range(NVT):
                pst = ps_tr.tile([64, 128], F32, tag="ps_tr")
                nc.tensor.transpose(pst[:], scoresT_sb[:, vt, :], identity[:])
                nc.vector.tensor_copy(scores_sb[:, vt * 128:(vt + 1) * 128], pst[:])

            neg_max = small.tile([64, 1], F32, tag="small")
            nc.vector.tensor_reduce(out=neg_max[:], in_=scores_sb[:],
                                    axis=AX.X, op=ALU.max, negate=True)
            return scores_sb, neg_max

        def emit_half_post(qt, h, prep):
            row0 = qt * 128 + h * 64
            scores_sb, neg_max = prep
            exp_sb = half_pool.tile([64, TV], F32, tag="exp")
            sums = small.tile([64, 1], F32, tag="small")
            nc.scalar.activation(out=exp_sb[:], in_=scores_sb[:],
                                 func=AF.Exp, bias=neg_max[:],
                                 accum_out=sums[:])
            recip = small.tile([64, 1], F32, tag="small")
            nc.vector.reciprocal(recip[:], sums[:])
            attn_sb = half_pool.tile([64, TV], F32, tag="attn")
            nc.vector.tensor_scalar_mul(attn_sb[:], exp_sb[:], recip[:])
            nc.sync.dma_start(out=attn_out[row0:row0 + 64, :], in_=attn_sb[:])

            # fp16 attn copy for the context matmul
            attn16_sb = half_pool.tile([64, TV], F16, tag="attn16")
            nc.vector.tensor_scalar_mul(attn16_sb[:], exp_sb[:], recip[:])
            attnT16_sb = half_pool.tile([128, NVT, 64], F16, tag="attnT")
            for vt in range(NVT):
                pst = ps_tr.tile([128, 64], F16, tag="ps_tr")
                nc.tensor.transpose(pst[:], attn16_sb[:, vt * 128:(vt + 1) * 128],
                                    identity16[0:64, 0:64])
                nc.vector.tensor_copy(attnT16_sb[:, vt, :], pst[:])

            ctx_ps = ps_tr.tile([64, D], F32, tag="ps_tr")
            for vt in range(NVT):
                nc.tensor.matmul(ctx_ps[:], attnT16_sb[:, vt, :],
                                 value16_sb[:, vt, :],
                                 start=(vt == 0), stop=(vt == NVT - 1))
            ctx_sb = half_pool.tile([64, D], F32, tag="ctx")
            nc.vector.tensor_copy(ctx_sb[:], ctx_ps[:])
            nc.sync.dma_start(out=ctx_out[row0:row0 + 64, :], in_=ctx_sb[:])

        def emit_half_softmax(qt, h, sc_ps):
            emit_half_post(qt, h, emit_half_prep(qt, h, sc_ps))

        halves = [(qt, h) for qt in range(NQT) for h in range(2)]
        prev = None
        for idx, (qt, h) in enumerate(halves):
            sc_ps = emit_half_scores(qt, h)
            last = idx == len(halves) - 1
            if last:
                # queue the final half's prep transposes on the PE before the
                # prior half's softmax block so the final exp's dependency
                # chain starts immediately after the last score matmul
                final_prep = emit_half_prep(qt, h, sc_ps)
            if prev is not None:
                emit_half_softmax(*prev)
            if last:
                emit_half_post(qt, h, final_prep)
            prev = (qt, h, sc_ps)

    nc.finalize()
    return nc


_program_cache: dict[str, bass.Bass] = {}


def _get_program() -> bass.Bass:
    if "nc" not in _program_cache:
        _program_cache["nc"] = build_program()
    return _program_cache["nc"]


def make_in_maps(query, value, W1, W2, scale):
    in_maps = []
    for c in range(NCORES):
        b = c // (NCORES // B)
        qc = c % (NCORES // B)
        qch = np.asarray(query[b, qc * QCH:(qc + 1) * QCH, :], dtype=np.float32)
        vb = np.asarray(value[b], dtype=np.float32)

        def tile_pmajor(a, p=128):
            # [N*p, M] -> [p, N, M] so the device sees one contiguous DMA
            n = a.shape[0] // p
            return np.ascontiguousarray(
                a.astype(np.float16).reshape(n, p, a.shape[1]).transpose(1, 0, 2))

        in_maps.append({
            "v16": tile_pmajor(vb),
            "vT16": tile_pmajor(vb.T),
            "qT16": tile_pmajor(qch.T),
            "w1_16": tile_pmajor(np.asarray(W1, np.float32)),
            "w2_16": tile_pmajor(np.asarray(W2, np.float32)),
            "s16": np.ascontiguousarray(
                np.asarray(scale, np.float32).reshape(U, 1).astype(np.float16)),
        })
    return in_maps


def assemble(results):
    ctx_full = np.empty((B, TQ, D), dtype=np.float32)
    attn_full = np.empty((B, TQ, TV), dtype=np.float32)
    for c in range(NCORES):
        b = c // (NCORES // B)
        qc = c % (NCORES // B)
        ctx_full[b, qc * QCH:(qc + 1) * QCH, :] = results[c]["ctx"]
        attn_full[b, qc * QCH:(qc + 1) * QCH, :] = results[c]["attn"]
    return ctx_full, attn_full


def kernel(query, value, W1, W2, scale):
    nc = _get_program()
    in_maps = make_in_maps(query, value, W1, W2, scale)
    res = run_bass_kernel_spmd(nc, in_maps, list(range(NCORES))).results
    return assemble(res)
